# revision 19
# baseline (speedup 1.0000x reference)
"""ALSTM cell (attention-augmented LSTM) on 8 TRN2 NeuronCores.

Strategy: data-parallel over batch (B=256 -> 32 per core), weights
replicated, sequential scan local per shard (no collectives).

Each core runs TWO independent 16-batch recurrence streams (A/B),
software-pipelined half a step apart, so one stream's PE matmul blocks
fill the other stream's serial softmax/pointwise dependency chain.
The PE cost of the weight-streaming matmuls is batch-size independent
(moving operand = weight columns), so 2x16 costs the same PE time as
1x32 but hides most of the chain latency.

Per-core layout: recurrent state kept TRANSPOSED
(hT/cT: [u_within_chunk(128part), kchunk, b]); gate matmuls col-tiled
(gate j stationary at PE columns 32j -> PSUM partitions 32j..32j+16),
which lets disjoint column-tile matmuls run concurrently on the PE
(32x32 sub-array col groups). Softmax normalization rides the
attention transpose matmul as a diag(1/sum) moving operand. Sigmoid
via tanh half-angle with doubled state (U matrices pre-halved on
host, output un-doubled on host). Biases are all zero in this spec.

v2 scheduling: every engine queue (PE/ACT/DVE) is in-order, so global
emission order is chosen so no queue head waits on a dep while ready
work sits behind it:
  PE:  frontB(t) | awtA ctxA(t) | xwaA(t+1) | awtB ctxB(t) | gtA(t)
       | gtB(t) | atthA+ghA(t+1)
  ACT: tanhB expB(t) | gactA | gactB | ctanhA | ctanhB | tanhA expA(t+1)
  DVE: ctxTA | recipB diagB | ctxTB | pwA | pwB | recipA diagA(t+1)
The xwaA(t+1) hoist fills the PE gap while gactA runs; B's back1 fills
A's pointwise window. gt transposes are emitted as regular matmuls
(gact.T @ I128 -> fp32 PSUM) so they count as normal PE busy work
(transpose-mode matmuls don't extend the PE's high-activity window).
"""

import sys

if "/opt/trn_rl_repo" not in sys.path:
    sys.path.append("/opt/trn_rl_repo")

from contextlib import ExitStack

import numpy as np

import concourse.bass as bass
import concourse.mybir as mybir
import concourse.tile as tile
from concourse.bass_utils import run_bass_kernel_spmd

F32 = mybir.dt.float32
BF16 = mybir.dt.bfloat16
AF = mybir.ActivationFunctionType

B, T, D, U = 256, 512, 256, 512
NCORES = 8
BS = B // NCORES  # 32 per core
BS2 = BS // 2  # 16 per stream
KU = U // 128  # 4 contraction chunks over h
KD = D // 128  # 2 contraction chunks over ctx/x
NG = 4  # gates i,f,o,c
USE_SLOTS = False  # force manual schedule via tile_wait_until slots
DUMMIES = True  # hot-filler matmuls at known PE stall sites


def _split_excess_waits(nc: bass.Bass, max_waits: int = 1) -> None:
    """Move excess semaphore waits onto standalone EventSemaphore
    instructions (the BIR form of wait_ge). walrus' per-instruction
    descriptor has room for only ~one sync wait; Tile sometimes attaches
    more (slot-reuse WAR/WAW across engines). Splitting is sound: the
    engine executes the preceding waits in stream order."""
    k = 0
    for fn in nc.m.functions:
        for blk in fn.blocks:
            out = []
            for inst in blk.instructions:
                si = inst.sync_info
                if si is not None and len(si.on_wait) > max_waits:
                    waits = list(si.on_wait)
                    for w in waits[:-max_waits]:
                        k += 1
                        out.append(
                            mybir.InstEventSemaphore(
                                name=f"xwait-{k}",
                                engine=inst.engine,
                                ins=[],
                                outs=[],
                                sync_info=mybir.SyncInfo(
                                    on_wait=[w], on_update=[]
                                ),
                            )
                        )
                    inst.sync_info = mybir.SyncInfo(
                        on_wait=waits[-max_waits:],
                        on_update=list(si.on_update),
                    )
                out.append(inst)
            blk.instructions = out


class Stream:
    """Per-stream (A/B) tiles and emission helpers."""

    def __init__(self, nc, tc, ctx, name, bofs, weights, out_dram):
        self.nc = nc
        self.name = name
        self.bofs = bofs  # 0 or BS2 within the shared x tile
        self.w = weights
        self.out_dram = out_dram

        st = ctx.enter_context(tc.tile_pool(name=f"st{name}", bufs=1))
        self.hT = st.tile([128, KU, BS2], BF16)
        nc.vector.memset(self.hT[:], 0.0)
        self.cT = st.tile([128, KU, BS2], F32)
        nc.vector.memset(self.cT[:], 0.0)

        self.ps_att = ctx.enter_context(
            tc.tile_pool(name=f"psa{name}", bufs=1, space="PSUM")
        )
        self.ps_g = ctx.enter_context(
            tc.tile_pool(name=f"psg{name}", bufs=1, space="PSUM")
        )
        self.ps_awt = ctx.enter_context(
            tc.tile_pool(name=f"psw{name}", bufs=1, space="PSUM")
        )
        self.ps_gt = ctx.enter_context(
            tc.tile_pool(name=f"pst{name}", bufs=1, space="PSUM")
        )
        self.smp = ctx.enter_context(tc.tile_pool(name=f"smp{name}", bufs=2))
        self.gp = ctx.enter_context(tc.tile_pool(name=f"gp{name}", bufs=2))
        self.hp = ctx.enter_context(tc.tile_pool(name=f"hp{name}", bufs=4))
        self.cp = ctx.enter_context(tc.tile_pool(name=f"cp{name}", bufs=2))

        # Zero the full gates psum bank once: gact reads all 128
        # partitions, only 32j..32j+16 are ever written by matmuls.
        g0 = self.ps_g.tile([128, U], F32, name=f"g{name}")
        nc.vector.memset(g0[:], 0.0)
        self.gates_ps = g0

    # ---- front phases (PE) ----

    def front_xwa(self, t, xt, stop=False):
        """x@Wa into att psum (h-independent, can hoist early)."""
        nc = self.nc
        att = self.ps_att.tile([BS2, D], F32, name=f"att{self.name}")
        self.att_ps = att
        xsl = slice(self.bofs, self.bofs + BS2)
        for kc in range(KD):
            nc.tensor.matmul(
                att[:],
                xt[:, kc, xsl],
                self.w["Wa"][:, kc, :],
                start=(kc == 0),
                stop=stop and (kc == KD - 1),
                skip_group_check=True,
            )

    def front_atth_gh(self, t, with_xwa_xt=None):
        """h@Ua (att psum) + h-part gate matmuls, woven so adjacent
        matmuls sit on different PE column tiles. If with_xwa_xt is
        given, also emits the xwa matmuls (combined front)."""
        nc = self.nc
        w = self.w
        gates = self.gates_ps
        att = None

        def atth(kc, stop):
            nc.tensor.matmul(
                att[:],
                self.hT[:, kc, :],
                w["Ua"][:, kc, :],
                start=False,
                stop=stop,
                skip_group_check=True,
            )

        def gh(g, kc):
            nc.tensor.matmul(
                gates[32 * g : 32 * g + BS2, :],
                self.hT[:, kc, :],
                w["Uall"][:, kc, 512 * g : 512 * (g + 1)],
                start=(kc == 0),
                stop=False,
                tile_position=(0, 32 * g),
            )

        if with_xwa_xt is not None:
            xt = with_xwa_xt
            xsl = slice(self.bofs, self.bofs + BS2)
            att_new = self.ps_att.tile([BS2, D], F32, name=f"att{self.name}")
            self.att_ps = att_new
            att = att_new

            def xwa(kc):
                nc.tensor.matmul(
                    att[:],
                    xt[:, kc, xsl],
                    self.w["Wa"][:, kc, :],
                    start=(kc == 0),
                    stop=False,
                    skip_group_check=True,
                )

            xwa(0)
            gh(1, 0)
            xwa(1)
            gh(2, 0)
            atth(0, False)
            gh(3, 0)
            atth(1, False)
            gh(1, 1)
            atth(2, False)
            gh(2, 1)
            atth(3, True)
            gh(3, 1)
            gh(0, 0)
            gh(1, 2)
            gh(0, 1)
            gh(2, 2)
            gh(0, 2)
            gh(3, 2)
            gh(0, 3)
            gh(1, 3)
            gh(2, 3)
            gh(3, 3)
        else:
            att = self.att_ps
            atth(0, False)
            gh(1, 0)
            atth(1, False)
            gh(2, 0)
            atth(2, False)
            gh(3, 0)
            atth(3, True)
            gh(1, 1)
            gh(0, 0)
            gh(2, 1)
            gh(0, 1)
            gh(3, 1)
            gh(0, 2)
            gh(1, 2)
            gh(2, 2)
            gh(3, 2)
            gh(0, 3)
            gh(1, 3)
            gh(2, 3)
            gh(3, 3)

    # ---- softmax (ACT then DVE) ----

    def soft_act(self, t):
        nc = self.nc
        att_t = self.smp.tile([BS2, D], F32, name=f"at{self.name}")
        nc.scalar.activation(att_t[:], self.att_ps[:], AF.Tanh)
        att_e = self.smp.tile([BS2, D], BF16, name=f"ae{self.name}")
        esum = self.smp.tile([BS2, 1], F32, name=f"es{self.name}")
        nc.scalar.activation(att_e[:], att_t[:], AF.Exp, accum_out=esum[:])
        self.att_e = att_e
        self.esum = esum

    def soft_dve(self, t):
        nc = self.nc
        rsum = self.smp.tile([BS2, 1], F32, name=f"rs{self.name}")
        nc.vector.reciprocal(rsum[:], self.esum[:])
        diag = self.smp.tile([BS2, BS2], BF16, name=f"dg{self.name}")
        nc.vector.tensor_scalar_mul(diag[:], self.w["I16"][:], rsum[:])
        self.diag = diag

    # ---- back1: eT transpose + ctx + ctx-part gate matmuls ----

    def back1_awt(self, t):
        nc = self.nc
        awtf = self.ps_awt.tile([128, 18, BS2], F32, name=f"aw{self.name}")
        awt = awtf[:, 0:KD, :]
        self.dummy_tgt = awtf[0:BS2, 2:18, :]  # [16,16,16] scratch
        for kc in range(KD):
            nc.tensor.matmul(
                awt[:, kc, :],
                self.att_e[:, 128 * kc : 128 * (kc + 1)],
                self.diag[:],
                start=True,
                stop=True,
            )
        self.awt = awt

    def dummy_mm(self, n=256):
        """Hot-filler matmul with no data deps: keeps the PE's activity
        window alive through a known dependency stall. Writes a scratch
        PSUM region nothing reads."""
        nc = self.nc
        for _ in range(max(1, n // 256)):
            nc.tensor.matmul(
                self.dummy_tgt,
                self.w["Wa"][:, 0, 0:BS2],
                self.w["Uall"][:, 0, 0:256],
                start=True,
                stop=True,
                skip_group_check=True,
            )

    def back1_ctxT(self, t, xtv):
        nc = self.nc
        ctxT = self.smp.tile([128, KD, BS2], BF16, name=f"cx{self.name}")
        xsl = slice(self.bofs, self.bofs + BS2)
        nc.vector.tensor_mul(ctxT[:], self.awt[:], xtv[:, :, xsl])
        self.ctxT = ctxT

    def back1_ctx_mm(self, t, kcs=None):
        nc = self.nc
        gates = self.gates_ps
        for kc in kcs if kcs is not None else range(KD):
            for g in (1, 2, 3, 0):
                nc.tensor.matmul(
                    gates[32 * g : 32 * g + BS2, :],
                    self.ctxT[:, kc, :],
                    self.w["Wall"][:, kc, 512 * g : 512 * (g + 1)],
                    start=False,
                    stop=(kc == KD - 1),
                    tile_position=(0, 32 * g),
                )

    # ---- back2: gate activation, transposes, pointwise ----

    def back2_gact(self, t, half=None):
        nc = self.nc
        if half is None or half == 0:
            gact = self.gp.tile([128, U], BF16, name=f"ga{self.name}")
            self.gact = gact
        gact = self.gact
        lo, hi = (0, U) if half is None else (half * U // 2, (half + 1) * U // 2)
        nc.scalar.activation(
            gact[:, lo:hi], self.gates_ps[:, lo:hi], AF.Tanh,
            scale=self.w["sc"][:],
        )

    def back2_gt(self, t, half=None):
        """Gate transposes as regular matmuls: gact_chunk.T @ I128."""
        nc = self.nc
        if half is None or half == 0:
            gt = self.ps_gt.tile([128, KU, 128], F32, name=f"gt{self.name}")
            self.gt = gt
            self.chp = gt[:, :, 96 : 96 + BS2]
        gt = self.gt
        ucs = range(KU) if half is None else range(half * KU // 2, (half + 1) * KU // 2)
        for uc in ucs:
            nc.tensor.matmul(
                gt[:, uc, :],
                self.gact[:, 128 * uc : 128 * (uc + 1)],
                self.w["I128"][:],
                start=True,
                stop=True,
            )

    def back2_pw_dve(self, t):
        nc = self.nc
        gt = self.gt
        iT = gt[:, :, 0:BS2]
        fT = gt[:, :, 32 : 32 + BS2]

        ch_sb = self.smp.tile([128, KU, BS2], BF16, name=f"ch{self.name}")
        nc.vector.tensor_copy(ch_sb[:], self.chp)
        t2 = self.smp.tile([128, KU, BS2], F32, name=f"t2{self.name}")
        nc.vector.scalar_tensor_tensor(
            t2[:], fT, 1.0, self.cT[:], mybir.AluOpType.add, mybir.AluOpType.mult
        )
        t1 = self.smp.tile([128, KU, BS2], F32, name=f"t1{self.name}")
        nc.vector.scalar_tensor_tensor(
            t1[:], iT, 1.0, ch_sb[:], mybir.AluOpType.add, mybir.AluOpType.mult
        )
        cT_new = self.cp.tile([128, KU, BS2], F32, name=f"c{self.name}")
        nc.vector.scalar_tensor_tensor(
            cT_new[:], t2[:], 0.5, t1[:], mybir.AluOpType.mult, mybir.AluOpType.add
        )
        self.cT = cT_new

    def back2_ctanh(self, t):
        nc = self.nc
        ctanh = self.smp.tile([128, KU, BS2], BF16, name=f"ct{self.name}")
        nc.scalar.activation(ctanh[:], self.cT[:], AF.Tanh, scale=0.5)
        self.ctanh = ctanh

    def back2_h(self, t):
        """h = (o+1)*tanh(c), split into kc halves: subtile deps let the
        next front's kc<2 matmuls start as soon as the first half lands."""
        nc = self.nc
        oT = self.gt[:, :, 64 : 64 + BS2]
        hT_new = self.hp.tile([128, KU, BS2], BF16, name=f"h{self.name}")
        half = KU // 2
        for lo, hi in ((0, half), (half, KU)):
            nc.vector.scalar_tensor_tensor(
                hT_new[:, lo:hi, :], oT[:, lo:hi, :], 1.0,
                self.ctanh[:, lo:hi, :], mybir.AluOpType.add,
                mybir.AluOpType.mult,
            )
        nc.sync.dma_start(self.out_dram[t], hT_new[:])
        self.hT = hT_new


def build_nc(t_steps: int = T) -> bass.Bass:
    nc = bass.Bass()
    xTt = nc.declare_dram_parameter("xTt", [t_steps, 128, KD, BS], BF16, isOutput=False)
    Uall = nc.declare_dram_parameter("Uall", [U, NG * U], BF16, isOutput=False)
    Wall = nc.declare_dram_parameter("Wall", [D, NG * U], BF16, isOutput=False)
    Ua = nc.declare_dram_parameter("Ua", [U, D], BF16, isOutput=False)
    Wa = nc.declare_dram_parameter("Wa", [D, D], BF16, isOutput=False)
    I16 = nc.declare_dram_parameter("I16", [BS2, BS2], BF16, isOutput=False)
    I128 = nc.declare_dram_parameter("I128", [128, 128], BF16, isOutput=False)
    outA = nc.declare_dram_parameter(
        "outA", [t_steps, 128, KU, BS2], BF16, isOutput=True
    )
    outB = nc.declare_dram_parameter(
        "outB", [t_steps, 128, KU, BS2], BF16, isOutput=True
    )

    with ExitStack() as ctx:
        tc = ctx.enter_context(tile.TileContext(nc))
        wp = ctx.enter_context(tc.tile_pool(name="wp", bufs=1))
        Uall_sb = wp.tile([128, KU, NG * U], BF16)
        for kc in range(KU):
            nc.sync.dma_start(Uall_sb[:, kc, :], Uall[128 * kc : 128 * (kc + 1), :])
        Wall_sb = wp.tile([128, KD, NG * U], BF16)
        for kc in range(KD):
            nc.sync.dma_start(Wall_sb[:, kc, :], Wall[128 * kc : 128 * (kc + 1), :])
        Ua_sb = wp.tile([128, KU, D], BF16)
        for kc in range(KU):
            nc.sync.dma_start(Ua_sb[:, kc, :], Ua[128 * kc : 128 * (kc + 1), :])
        Wa_sb = wp.tile([128, KD, D], BF16)
        for kc in range(KD):
            nc.sync.dma_start(Wa_sb[:, kc, :], Wa[128 * kc : 128 * (kc + 1), :])
        I16_sb = wp.tile([BS2, BS2], BF16)
        nc.sync.dma_start(I16_sb[:], I16[:])
        I128_sb = wp.tile([128, 128], BF16)
        nc.sync.dma_start(I128_sb[:], I128[:])

        st = ctx.enter_context(tc.tile_pool(name="st", bufs=1))
        sc = st.tile([128, 1], F32)
        nc.vector.memset(sc[0:96, :], 0.5)
        nc.vector.memset(sc[96:128, :], 1.0)

        weights = {
            "Uall": Uall_sb,
            "Wall": Wall_sb,
            "Ua": Ua_sb,
            "Wa": Wa_sb,
            "I16": I16_sb,
            "I128": I128_sb,
            "sc": sc,
        }

        xp = ctx.enter_context(tc.tile_pool(name="xp", bufs=4))

        A = Stream(nc, tc, ctx, "A", 0, weights, outA)
        Bs = Stream(nc, tc, ctx, "B", BS2, weights, outB)

        def xdma(t):
            # shared x_t tiles: one copy for the PE (xWa lhsT), one for
            # the DVE (ctxT multiply) to keep DMA WAR fan-in small.
            xt = xp.tile([128, KD, BS], BF16, name="xt")
            nc.sync.dma_start(xt[:], xTt[t])
            xtv = xp.tile([128, KD, BS], BF16, name="xtv")
            nc.sync.dma_start(xtv[:], xTt[t])
            return xt, xtv

        # Software pipeline: A runs step t+1's front inside iteration t.
        # Every phase is pinned to a virtual scheduler slot via
        # tile_wait_until so the Tile scheduler emits exactly this
        # per-engine program order (its cost-model sim does not know
        # about PE column-tile concurrency, so its own greedy order is
        # wrong for this kernel).
        NSLOT = 22

        import contextlib

        def slot(t, k):
            if USE_SLOTS:
                return tc.tile_wait_until(t * NSLOT + k)
            return contextlib.nullcontext()

        xts = {0: xdma(0), 1: xdma(1)}
        # Prologue: A's front(0) + soft(0) so iteration 0 can do A.back1;
        # B's xwa(0) (normally hoisted into the previous iteration).
        A.front_xwa(0, xts[0][0])
        A.front_atth_gh(0)
        A.soft_act(0)
        A.soft_dve(0)
        Bs.front_xwa(0, xts[0][0])
        for tt in range(t_steps):
            t = tt + 1  # slot epoch (prologue used epoch 0)
            if tt + 2 < t_steps:
                with slot(t, 0):
                    xts[tt + 2] = xdma(tt + 2)
            # PE: B.front(t) rest (atth+gh woven; xwa was hoisted)
            with slot(t, 0):
                Bs.front_atth_gh(tt)
            # ACT: B soft early so its queue head never blocks
            with slot(t, 1):
                Bs.soft_act(tt)
            # PE: A back1 awt, then xwa_A(t+1) fills the ctxT_A bounce
            with slot(t, 2):
                A.back1_awt(tt)
            if tt + 1 < t_steps:
                with slot(t, 3):
                    A.front_xwa(tt + 1, xts[tt + 1][0])
            with slot(t, 4):
                A.back1_ctxT(tt, xts[tt][1])
            with slot(t, 5):
                A.back1_ctx_mm(tt, kcs=[0])
            # DVE: B soft tail
            with slot(t, 6):
                Bs.soft_dve(tt)
            # PE: awt_B interleaved into the middle of ctx_A
            with slot(t, 7):
                Bs.back1_awt(tt)
            with slot(t, 8):
                A.back1_ctx_mm(tt, kcs=[1])
            # PE: xwa_B(t+1) fills the ctxT_B bounce
            if tt + 1 < t_steps:
                with slot(t, 9):
                    Bs.front_xwa(tt + 1, xts[tt + 1][0])
            with slot(t, 10):
                Bs.back1_ctxT(tt, xts[tt][1])
            with slot(t, 11):
                Bs.back1_ctx_mm(tt)
            # ACT: gates activations in u-halves, transposes pipelined
            with slot(t, 12):
                A.back2_gact(tt, 0)
                Bs.back2_gact(tt, 0)
            if DUMMIES:
                with slot(t, 14):
                    A.dummy_mm(256)
            with slot(t, 14):
                A.back2_gt(tt, 0)
            with slot(t, 13):
                A.back2_gact(tt, 1)
                Bs.back2_gact(tt, 1)
            with slot(t, 14):
                A.back2_gt(tt, 1)
            if DUMMIES:
                with slot(t, 15):
                    Bs.dummy_mm(256)
            with slot(t, 15):
                Bs.back2_gt(tt)
            # DVE+ACT: A pointwise -> h_A(t+1)
            with slot(t, 16):
                A.back2_pw_dve(tt)
                A.back2_ctanh(tt)
                A.back2_h(tt)
            # PE: A front rest (t+1) — fills B's pointwise window
            if DUMMIES:
                with slot(t, 17):
                    A.dummy_mm(256)
                    Bs.dummy_mm(256)
            if tt + 1 < t_steps:
                with slot(t, 17):
                    A.front_atth_gh(tt + 1)
            # DVE+ACT: B pointwise -> h_B(t+1)
            with slot(t, 18):
                Bs.back2_pw_dve(tt)
                Bs.back2_ctanh(tt)
                Bs.back2_h(tt)
            # ACT+DVE: A soft(t+1)
            if tt + 1 < t_steps:
                with slot(t, 19):
                    A.soft_act(tt + 1)
                with slot(t, 20):
                    A.soft_dve(tt + 1)
            xts.pop(tt)

    _split_excess_waits(nc)
    return nc


def make_in_maps(x, W_i, U_i, W_f, U_f, W_o, U_o, W_c, U_c, W_a, U_a, t_steps=T):
    import ml_dtypes

    bf = ml_dtypes.bfloat16
    Uall = np.ascontiguousarray(
        np.concatenate([U_i, U_f, U_o, U_c], axis=1) * 0.5, bf
    )
    Wall = np.ascontiguousarray(np.concatenate([W_i, W_f, W_o, W_c], axis=1), bf)
    I16 = np.eye(BS2, dtype=bf)
    I128 = np.eye(128, dtype=bf)
    in_maps = []
    for i in range(NCORES):
        xs = np.asarray(x[BS * i : BS * (i + 1), :t_steps])  # [32, T, 256]
        # xTt[t, p, kc, b] = x[b, t, kc*128 + p]
        xTt = np.ascontiguousarray(
            xs.transpose(1, 2, 0).reshape(t_steps, KD, 128, BS).transpose(0, 2, 1, 3),
            bf,
        )
        in_maps.append(
            {
                "xTt": xTt,
                "Uall": Uall,
                "Wall": Wall,
                "Ua": np.ascontiguousarray(U_a * 0.5, bf),
                "Wa": np.ascontiguousarray(W_a, bf),
                "I16": I16,
                "I128": I128,
            }
        )
    return in_maps


def run(inputs, t_steps=T, trace=False, **spmd_kwargs):
    nc = build_nc(t_steps)
    in_maps = make_in_maps(
        inputs["x"],
        inputs["W_i"], inputs["U_i"],
        inputs["W_f"], inputs["U_f"],
        inputs["W_o"], inputs["U_o"],
        inputs["W_c"], inputs["U_c"],
        inputs["W_a"], inputs["U_a"],
        t_steps=t_steps,
    )
    res = run_bass_kernel_spmd(
        nc, in_maps, core_ids=list(range(NCORES)), trace=trace, **spmd_kwargs
    )
    outs = []
    for r in res.results:
        # out[t, p, uc, b] holds 2*h; u = uc*128 + p
        blocks = []
        for key in ("outA", "outB"):
            o = np.asarray(r[key]).astype(np.float32) * 0.5
            o = o.transpose(3, 0, 2, 1).reshape(BS2, t_steps, U)
            blocks.append(o)
        outs.append(np.concatenate(blocks, axis=0))  # [32, T, U]
    full = np.concatenate(outs, axis=0)
    return full, res


def kernel(**inputs) -> np.ndarray:
    full, _ = run(inputs)
    return full.astype(np.float32)


# revision 23
# speedup vs baseline: 1.0303x; 1.0303x over previous
"""ALSTM cell (attention-augmented LSTM) on 8 TRN2 NeuronCores.

Strategy: data-parallel over batch (B=256 -> 32 per core), weights
replicated, sequential scan local per shard (no collectives).

Each core runs TWO independent 16-batch recurrence streams (A/B),
software-pipelined half a step apart, so one stream's PE matmul blocks
fill the other stream's serial softmax/pointwise dependency chain.
The PE cost of the weight-streaming matmuls is batch-size independent
(moving operand = weight columns), so 2x16 costs the same PE time as
1x32 but hides most of the chain latency.

Per-core layout: recurrent state kept TRANSPOSED
(hT/cT: [u_within_chunk(128part), kchunk, b]); gate matmuls col-tiled
(gate j stationary at PE columns 32j -> PSUM partitions 32j..32j+16),
which lets disjoint column-tile matmuls run concurrently on the PE
(32x32 sub-array col groups). Softmax normalization rides the
attention transpose matmul as a diag(1/sum) moving operand. Sigmoid
via tanh half-angle with doubled state (U matrices pre-halved on
host, output un-doubled on host). Biases are all zero in this spec.

v2 scheduling: every engine queue (PE/ACT/DVE) is in-order, so global
emission order is chosen so no queue head waits on a dep while ready
work sits behind it:
  PE:  frontB(t) | awtA ctxA(t) | xwaA(t+1) | awtB ctxB(t) | gtA(t)
       | gtB(t) | atthA+ghA(t+1)
  ACT: tanhB expB(t) | gactA | gactB | ctanhA | ctanhB | tanhA expA(t+1)
  DVE: ctxTA | recipB diagB | ctxTB | pwA | pwB | recipA diagA(t+1)
The xwaA(t+1) hoist fills the PE gap while gactA runs; B's back1 fills
A's pointwise window. gt transposes are emitted as regular matmuls
(gact.T @ I128 -> fp32 PSUM) so they count as normal PE busy work
(transpose-mode matmuls don't extend the PE's high-activity window).
"""

import sys

if "/opt/trn_rl_repo" not in sys.path:
    sys.path.append("/opt/trn_rl_repo")

from contextlib import ExitStack

import numpy as np

import concourse.bass as bass
import concourse.mybir as mybir
import concourse.tile as tile
from concourse.bass_utils import run_bass_kernel_spmd

F32 = mybir.dt.float32
BF16 = mybir.dt.bfloat16
AF = mybir.ActivationFunctionType

B, T, D, U = 256, 512, 256, 512
NCORES = 8
BS = B // NCORES  # 32 per core
BS2 = BS // 2  # 16 per stream
KU = U // 128  # 4 contraction chunks over h
KD = D // 128  # 2 contraction chunks over ctx/x
NG = 4  # gates i,f,o,c
USE_SLOTS = False  # force manual schedule via tile_wait_until slots
DUMMIES = True  # hot-filler matmuls at known PE stall sites


def _split_excess_waits(nc: bass.Bass, max_waits: int = 1) -> None:
    """Move excess semaphore waits onto standalone EventSemaphore
    instructions (the BIR form of wait_ge). walrus' per-instruction
    descriptor has room for only ~one sync wait; Tile sometimes attaches
    more (slot-reuse WAR/WAW across engines). Splitting is sound: the
    engine executes the preceding waits in stream order."""
    k = 0
    for fn in nc.m.functions:
        for blk in fn.blocks:
            out = []
            for inst in blk.instructions:
                si = inst.sync_info
                if si is not None and len(si.on_wait) > max_waits:
                    waits = list(si.on_wait)
                    for w in waits[:-max_waits]:
                        k += 1
                        out.append(
                            mybir.InstEventSemaphore(
                                name=f"xwait-{k}",
                                engine=inst.engine,
                                ins=[],
                                outs=[],
                                sync_info=mybir.SyncInfo(
                                    on_wait=[w], on_update=[]
                                ),
                            )
                        )
                    inst.sync_info = mybir.SyncInfo(
                        on_wait=waits[-max_waits:],
                        on_update=list(si.on_update),
                    )
                out.append(inst)
            blk.instructions = out


class Stream:
    """Per-stream (A/B) tiles and emission helpers."""

    def __init__(self, nc, tc, ctx, name, bofs, weights, out_dram):
        self.nc = nc
        self.name = name
        self.bofs = bofs  # 0 or BS2 within the shared x tile
        self.w = weights
        self.out_dram = out_dram

        st = ctx.enter_context(tc.tile_pool(name=f"st{name}", bufs=1))
        self.hT = st.tile([128, KU, BS2], BF16)
        nc.vector.memset(self.hT[:], 0.0)
        self.cT = st.tile([128, KU, BS2], F32)
        nc.vector.memset(self.cT[:], 0.0)

        self.ps_att = ctx.enter_context(
            tc.tile_pool(name=f"psa{name}", bufs=1, space="PSUM")
        )
        self.ps_g = ctx.enter_context(
            tc.tile_pool(name=f"psg{name}", bufs=1, space="PSUM")
        )
        self.ps_awt = ctx.enter_context(
            tc.tile_pool(name=f"psw{name}", bufs=1, space="PSUM")
        )
        self.ps_gt = ctx.enter_context(
            tc.tile_pool(name=f"pst{name}", bufs=1, space="PSUM")
        )
        self.smp = ctx.enter_context(tc.tile_pool(name=f"smp{name}", bufs=2))
        self.gp = ctx.enter_context(tc.tile_pool(name=f"gp{name}", bufs=2))
        self.hp = ctx.enter_context(tc.tile_pool(name=f"hp{name}", bufs=4))
        self.cp = ctx.enter_context(tc.tile_pool(name=f"cp{name}", bufs=2))

        # Zero the full gates psum bank once: gact reads all 128
        # partitions, only 32j..32j+16 are ever written by matmuls.
        g0 = self.ps_g.tile([128, U], F32, name=f"g{name}")
        nc.vector.memset(g0[:], 0.0)
        self.gates_ps = g0

    # ---- front phases (PE) ----

    def front_xwa(self, t, xt, stop=False):
        """x@Wa into att psum (h-independent, can hoist early)."""
        nc = self.nc
        att = self.ps_att.tile([BS2, D], F32, name=f"att{self.name}")
        self.att_ps = att
        xsl = slice(self.bofs, self.bofs + BS2)
        for kc in range(KD):
            nc.tensor.matmul(
                att[:],
                xt[:, kc, xsl],
                self.w["Wa"][:, kc, :],
                start=(kc == 0),
                stop=stop and (kc == KD - 1),
                skip_group_check=True,
            )

    def front_atth_gh(self, t, with_xwa_xt=None):
        """h@Ua (att psum) + h-part gate matmuls, woven so adjacent
        matmuls sit on different PE column tiles. If with_xwa_xt is
        given, also emits the xwa matmuls (combined front)."""
        nc = self.nc
        w = self.w
        gates = self.gates_ps
        att = None

        def atth(kc, stop):
            nc.tensor.matmul(
                att[:],
                self.hT[:, kc, :],
                w["Ua"][:, kc, :],
                start=False,
                stop=stop,
                skip_group_check=True,
            )

        def gh(g, kc):
            nc.tensor.matmul(
                gates[32 * g : 32 * g + BS2, :],
                self.hT[:, kc, :],
                w["Uall"][:, kc, 512 * g : 512 * (g + 1)],
                start=(kc == 0),
                stop=False,
                tile_position=(0, 32 * g),
            )

        if with_xwa_xt is not None:
            xt = with_xwa_xt
            xsl = slice(self.bofs, self.bofs + BS2)
            att_new = self.ps_att.tile([BS2, D], F32, name=f"att{self.name}")
            self.att_ps = att_new
            att = att_new

            def xwa(kc):
                nc.tensor.matmul(
                    att[:],
                    xt[:, kc, xsl],
                    self.w["Wa"][:, kc, :],
                    start=(kc == 0),
                    stop=False,
                    skip_group_check=True,
                )

            xwa(0)
            gh(1, 0)
            xwa(1)
            gh(2, 0)
            atth(0, False)
            gh(3, 0)
            atth(1, False)
            gh(1, 1)
            atth(2, False)
            gh(2, 1)
            atth(3, True)
            gh(3, 1)
            gh(0, 0)
            gh(1, 2)
            gh(0, 1)
            gh(2, 2)
            gh(0, 2)
            gh(3, 2)
            gh(0, 3)
            gh(1, 3)
            gh(2, 3)
            gh(3, 3)
        else:
            att = self.att_ps
            atth(0, False)
            gh(1, 0)
            atth(1, False)
            gh(2, 0)
            atth(2, False)
            gh(3, 0)
            atth(3, True)
            gh(1, 1)
            gh(0, 0)
            gh(2, 1)
            gh(0, 1)
            gh(3, 1)
            gh(0, 2)
            gh(1, 2)
            gh(2, 2)
            gh(3, 2)
            gh(0, 3)
            gh(1, 3)
            gh(2, 3)
            gh(3, 3)

    # ---- softmax (ACT then DVE) ----

    def soft_act(self, t):
        nc = self.nc
        att_t = self.smp.tile([BS2, D], F32, name=f"at{self.name}")
        nc.scalar.activation(att_t[:], self.att_ps[:], AF.Tanh)
        att_e = self.smp.tile([BS2, D], BF16, name=f"ae{self.name}")
        esum = self.smp.tile([BS2, 1], F32, name=f"es{self.name}")
        nc.scalar.activation(att_e[:], att_t[:], AF.Exp, accum_out=esum[:])
        self.att_e = att_e
        self.esum = esum

    def soft_dve(self, t):
        nc = self.nc
        rsum = self.smp.tile([BS2, 1], F32, name=f"rs{self.name}")
        nc.vector.reciprocal(rsum[:], self.esum[:])
        diag = self.smp.tile([BS2, BS2], BF16, name=f"dg{self.name}")
        nc.vector.tensor_scalar_mul(diag[:], self.w["I16"][:], rsum[:])
        self.diag = diag

    # ---- back1: eT transpose + ctx + ctx-part gate matmuls ----

    def back1_awt(self, t):
        nc = self.nc
        awtf = self.ps_awt.tile([128, 18, BS2], F32, name=f"aw{self.name}")
        awt = awtf[:, 0:KD, :]
        self.dummy_tgt = awtf[0:BS2, 2:18, :]  # [16,16,16] scratch
        for kc in range(KD):
            nc.tensor.matmul(
                awt[:, kc, :],
                self.att_e[:, 128 * kc : 128 * (kc + 1)],
                self.diag[:],
                start=True,
                stop=True,
            )
        self.awt = awt

    def dummy_mm(self, n=256):
        """Hot-filler matmul with no data deps: keeps the PE's activity
        window alive through a known dependency stall. Writes a scratch
        PSUM region nothing reads."""
        nc = self.nc
        for _ in range(max(1, n // 256)):
            nc.tensor.matmul(
                self.dummy_tgt,
                self.w["Wa"][:, 0, 0:BS2],
                self.w["Uall"][:, 0, 0:256],
                start=True,
                stop=True,
                skip_group_check=True,
            )

    def back1_ctxT(self, t, xtv):
        nc = self.nc
        ctxT = self.smp.tile([128, KD, BS2], BF16, name=f"cx{self.name}")
        xsl = slice(self.bofs, self.bofs + BS2)
        nc.vector.tensor_mul(ctxT[:], self.awt[:], xtv[:, :, xsl])
        self.ctxT = ctxT

    def back1_ctx_mm(self, t, kcs=None):
        nc = self.nc
        gates = self.gates_ps
        for kc in kcs if kcs is not None else range(KD):
            for g in (1, 2, 3, 0):
                nc.tensor.matmul(
                    gates[32 * g : 32 * g + BS2, :],
                    self.ctxT[:, kc, :],
                    self.w["Wall"][:, kc, 512 * g : 512 * (g + 1)],
                    start=False,
                    stop=(kc == KD - 1),
                    tile_position=(0, 32 * g),
                )

    # ---- back2: gate activation, transposes, pointwise ----

    def back2_gact(self, t):
        nc = self.nc
        gact = self.gp.tile([128, U], BF16, name=f"ga{self.name}")
        nc.scalar.activation(
            gact[:], self.gates_ps[:], AF.Tanh, scale=self.w["sc"][:]
        )
        self.gact = gact

    def back2_gt(self, t):
        """Gate transposes as regular matmuls: gact_chunk.T @ I128."""
        nc = self.nc
        gt = self.ps_gt.tile([128, KU, 128], F32, name=f"gt{self.name}")
        for uc in range(KU):
            nc.tensor.matmul(
                gt[:, uc, :],
                self.gact[:, 128 * uc : 128 * (uc + 1)],
                self.w["I128"][:],
                start=True,
                stop=True,
            )
        self.gt = gt
        self.chp = gt[:, :, 96 : 96 + BS2]

    def back2_pw_dve(self, t):
        nc = self.nc
        gt = self.gt
        iT = gt[:, :, 0:BS2]
        fT = gt[:, :, 32 : 32 + BS2]

        ch_sb = self.smp.tile([128, KU, BS2], BF16, name=f"ch{self.name}")
        nc.vector.tensor_copy(ch_sb[:], self.chp)
        t2 = self.smp.tile([128, KU, BS2], F32, name=f"t2{self.name}")
        nc.vector.scalar_tensor_tensor(
            t2[:], fT, 1.0, self.cT[:], mybir.AluOpType.add, mybir.AluOpType.mult
        )
        t1 = self.smp.tile([128, KU, BS2], F32, name=f"t1{self.name}")
        nc.vector.scalar_tensor_tensor(
            t1[:], iT, 1.0, ch_sb[:], mybir.AluOpType.add, mybir.AluOpType.mult
        )
        cT_new = self.cp.tile([128, KU, BS2], F32, name=f"c{self.name}")
        nc.vector.scalar_tensor_tensor(
            cT_new[:], t2[:], 0.5, t1[:], mybir.AluOpType.mult, mybir.AluOpType.add
        )
        self.cT = cT_new

    def back2_ctanh(self, t):
        nc = self.nc
        ctanh = self.smp.tile([128, KU, BS2], BF16, name=f"ct{self.name}")
        nc.scalar.activation(ctanh[:], self.cT[:], AF.Tanh, scale=0.5)
        self.ctanh = ctanh

    def back2_h(self, t):
        """h = (o+1)*tanh(c), split into kc halves: subtile deps let the
        next front's kc<2 matmuls start as soon as the first half lands."""
        nc = self.nc
        oT = self.gt[:, :, 64 : 64 + BS2]
        hT_new = self.hp.tile([128, KU, BS2], BF16, name=f"h{self.name}")
        half = KU // 2
        for lo, hi in ((0, half), (half, KU)):
            nc.vector.scalar_tensor_tensor(
                hT_new[:, lo:hi, :], oT[:, lo:hi, :], 1.0,
                self.ctanh[:, lo:hi, :], mybir.AluOpType.add,
                mybir.AluOpType.mult,
            )
        nc.sync.dma_start(self.out_dram[t], hT_new[:])
        self.hT = hT_new


def build_nc(t_steps: int = T) -> bass.Bass:
    nc = bass.Bass()
    xTt = nc.declare_dram_parameter("xTt", [t_steps, 128, KD, BS], BF16, isOutput=False)
    Uall = nc.declare_dram_parameter("Uall", [U, NG * U], BF16, isOutput=False)
    Wall = nc.declare_dram_parameter("Wall", [D, NG * U], BF16, isOutput=False)
    Ua = nc.declare_dram_parameter("Ua", [U, D], BF16, isOutput=False)
    Wa = nc.declare_dram_parameter("Wa", [D, D], BF16, isOutput=False)
    I16 = nc.declare_dram_parameter("I16", [BS2, BS2], BF16, isOutput=False)
    I128 = nc.declare_dram_parameter("I128", [128, 128], BF16, isOutput=False)
    outA = nc.declare_dram_parameter(
        "outA", [t_steps, 128, KU, BS2], BF16, isOutput=True
    )
    outB = nc.declare_dram_parameter(
        "outB", [t_steps, 128, KU, BS2], BF16, isOutput=True
    )

    with ExitStack() as ctx:
        tc = ctx.enter_context(tile.TileContext(nc))
        wp = ctx.enter_context(tc.tile_pool(name="wp", bufs=1))
        Uall_sb = wp.tile([128, KU, NG * U], BF16)
        for kc in range(KU):
            nc.sync.dma_start(Uall_sb[:, kc, :], Uall[128 * kc : 128 * (kc + 1), :])
        Wall_sb = wp.tile([128, KD, NG * U], BF16)
        for kc in range(KD):
            nc.sync.dma_start(Wall_sb[:, kc, :], Wall[128 * kc : 128 * (kc + 1), :])
        Ua_sb = wp.tile([128, KU, D], BF16)
        for kc in range(KU):
            nc.sync.dma_start(Ua_sb[:, kc, :], Ua[128 * kc : 128 * (kc + 1), :])
        Wa_sb = wp.tile([128, KD, D], BF16)
        for kc in range(KD):
            nc.sync.dma_start(Wa_sb[:, kc, :], Wa[128 * kc : 128 * (kc + 1), :])
        I16_sb = wp.tile([BS2, BS2], BF16)
        nc.sync.dma_start(I16_sb[:], I16[:])
        I128_sb = wp.tile([128, 128], BF16)
        nc.sync.dma_start(I128_sb[:], I128[:])

        st = ctx.enter_context(tc.tile_pool(name="st", bufs=1))
        sc = st.tile([128, 1], F32)
        nc.vector.memset(sc[0:96, :], 0.5)
        nc.vector.memset(sc[96:128, :], 1.0)

        weights = {
            "Uall": Uall_sb,
            "Wall": Wall_sb,
            "Ua": Ua_sb,
            "Wa": Wa_sb,
            "I16": I16_sb,
            "I128": I128_sb,
            "sc": sc,
        }

        xp = ctx.enter_context(tc.tile_pool(name="xp", bufs=4))

        A = Stream(nc, tc, ctx, "A", 0, weights, outA)
        Bs = Stream(nc, tc, ctx, "B", BS2, weights, outB)

        def xdma(t):
            # shared x_t tiles: one copy for the PE (xWa lhsT), one for
            # the DVE (ctxT multiply) to keep DMA WAR fan-in small.
            xt = xp.tile([128, KD, BS], BF16, name="xt")
            nc.sync.dma_start(xt[:], xTt[t])
            xtv = xp.tile([128, KD, BS], BF16, name="xtv")
            nc.sync.dma_start(xtv[:], xTt[t])
            return xt, xtv

        # Software pipeline: A runs step t+1's front inside iteration t.
        # Every phase is pinned to a virtual scheduler slot via
        # tile_wait_until so the Tile scheduler emits exactly this
        # per-engine program order (its cost-model sim does not know
        # about PE column-tile concurrency, so its own greedy order is
        # wrong for this kernel).
        NSLOT = 22

        import contextlib

        def slot(t, k):
            if USE_SLOTS:
                return tc.tile_wait_until(t * NSLOT + k)
            return contextlib.nullcontext()

        xts = {0: xdma(0), 1: xdma(1)}
        # Prologue: A's front(0) + soft(0) so iteration 0 can do A.back1;
        # B's xwa(0) (normally hoisted into the previous iteration).
        A.front_xwa(0, xts[0][0])
        A.front_atth_gh(0)
        A.soft_act(0)
        A.soft_dve(0)
        Bs.front_xwa(0, xts[0][0])
        for tt in range(t_steps):
            t = tt + 1  # slot epoch (prologue used epoch 0)
            if tt + 2 < t_steps:
                with slot(t, 0):
                    xts[tt + 2] = xdma(tt + 2)
            # PE: B.front(t) rest (atth+gh woven; xwa was hoisted)
            with slot(t, 0):
                Bs.front_atth_gh(tt)
            # ACT: B soft early so its queue head never blocks
            with slot(t, 1):
                Bs.soft_act(tt)
            # PE: A back1 awt, then xwa_A(t+1) fills the ctxT_A bounce
            with slot(t, 2):
                A.back1_awt(tt)
            if tt + 1 < t_steps:
                with slot(t, 3):
                    A.front_xwa(tt + 1, xts[tt + 1][0])
            with slot(t, 4):
                A.back1_ctxT(tt, xts[tt][1])
            with slot(t, 5):
                A.back1_ctx_mm(tt, kcs=[0])
            # DVE: B soft tail
            with slot(t, 6):
                Bs.soft_dve(tt)
            # PE: awt_B interleaved into the middle of ctx_A
            with slot(t, 7):
                Bs.back1_awt(tt)
            with slot(t, 8):
                A.back1_ctx_mm(tt, kcs=[1])
            # PE: xwa_B(t+1) fills the ctxT_B bounce
            if tt + 1 < t_steps:
                with slot(t, 9):
                    Bs.front_xwa(tt + 1, xts[tt + 1][0])
            with slot(t, 10):
                Bs.back1_ctxT(tt, xts[tt][1])
            with slot(t, 11):
                Bs.back1_ctx_mm(tt)
            # ACT: gates activations back-to-back
            with slot(t, 12):
                A.back2_gact(tt)
            with slot(t, 13):
                Bs.back2_gact(tt)
            # PE: gate transposes (with hot-filler dummies ahead of the
            # gact waits when enabled)
            if DUMMIES:
                with slot(t, 14):
                    A.dummy_mm(256)
            with slot(t, 14):
                A.back2_gt(tt)
            if DUMMIES:
                with slot(t, 15):
                    Bs.dummy_mm(256)
            with slot(t, 15):
                Bs.back2_gt(tt)
            # DVE+ACT: A pointwise -> h_A(t+1)
            with slot(t, 16):
                A.back2_pw_dve(tt)
                A.back2_ctanh(tt)
                A.back2_h(tt)
            # PE: A front rest (t+1) — fills B's pointwise window
            if DUMMIES:
                with slot(t, 17):
                    A.dummy_mm(256)
                    Bs.dummy_mm(256)
                    A.dummy_mm(256)
            if tt + 1 < t_steps:
                with slot(t, 17):
                    A.front_atth_gh(tt + 1)
            # DVE+ACT: B pointwise -> h_B(t+1)
            with slot(t, 18):
                Bs.back2_pw_dve(tt)
                Bs.back2_ctanh(tt)
                Bs.back2_h(tt)
            # ACT+DVE: A soft(t+1)
            if tt + 1 < t_steps:
                with slot(t, 19):
                    A.soft_act(tt + 1)
                with slot(t, 20):
                    A.soft_dve(tt + 1)
            xts.pop(tt)

    _split_excess_waits(nc)
    return nc


def make_in_maps(x, W_i, U_i, W_f, U_f, W_o, U_o, W_c, U_c, W_a, U_a, t_steps=T):
    import ml_dtypes

    bf = ml_dtypes.bfloat16
    Uall = np.ascontiguousarray(
        np.concatenate([U_i, U_f, U_o, U_c], axis=1) * 0.5, bf
    )
    Wall = np.ascontiguousarray(np.concatenate([W_i, W_f, W_o, W_c], axis=1), bf)
    I16 = np.eye(BS2, dtype=bf)
    I128 = np.eye(128, dtype=bf)
    in_maps = []
    for i in range(NCORES):
        xs = np.asarray(x[BS * i : BS * (i + 1), :t_steps])  # [32, T, 256]
        # xTt[t, p, kc, b] = x[b, t, kc*128 + p]
        xTt = np.ascontiguousarray(
            xs.transpose(1, 2, 0).reshape(t_steps, KD, 128, BS).transpose(0, 2, 1, 3),
            bf,
        )
        in_maps.append(
            {
                "xTt": xTt,
                "Uall": Uall,
                "Wall": Wall,
                "Ua": np.ascontiguousarray(U_a * 0.5, bf),
                "Wa": np.ascontiguousarray(W_a, bf),
                "I16": I16,
                "I128": I128,
            }
        )
    return in_maps


def run(inputs, t_steps=T, trace=False, **spmd_kwargs):
    nc = build_nc(t_steps)
    in_maps = make_in_maps(
        inputs["x"],
        inputs["W_i"], inputs["U_i"],
        inputs["W_f"], inputs["U_f"],
        inputs["W_o"], inputs["U_o"],
        inputs["W_c"], inputs["U_c"],
        inputs["W_a"], inputs["U_a"],
        t_steps=t_steps,
    )
    res = run_bass_kernel_spmd(
        nc, in_maps, core_ids=list(range(NCORES)), trace=trace, **spmd_kwargs
    )
    outs = []
    for r in res.results:
        # out[t, p, uc, b] holds 2*h; u = uc*128 + p
        blocks = []
        for key in ("outA", "outB"):
            o = np.asarray(r[key]).astype(np.float32) * 0.5
            o = o.transpose(3, 0, 2, 1).reshape(BS2, t_steps, U)
            blocks.append(o)
        outs.append(np.concatenate(blocks, axis=0))  # [32, T, U]
    full = np.concatenate(outs, axis=0)
    return full, res


def kernel(**inputs) -> np.ndarray:
    full, _ = run(inputs)
    return full.astype(np.float32)
